# revision 2
# baseline (speedup 1.0000x reference)
"""MoE FFN (DeepSeek-style top-2 routing + shared expert) on 8 TRN2 cores.

Sharding: expert-parallel for the 8 routed experts (core e owns expert e,
host gathers/pads its top-2 tokens to a fixed capacity C); the shared
expert is split 2 token-halves x 4 F-quarters (384 F-rows each) so its
weights stay tiny and SBUF-resident. Host does router + dispatch/combine
(the unshard step); device does all FLOPs-heavy matmuls.

Self-contained: hardcodes B=2,S=2048,D=768,E=8,K=2,F=1536.
"""
import ml_dtypes
import numpy as np
from contextlib import ExitStack

import concourse.bacc as bacc
import concourse.mybir as mybir
import concourse.tile as tile
from concourse.bass import ts
from concourse.bass_utils import run_bass_kernel_spmd

B, S, D = 2, 2048, 768
E, TOPK, F = 8, 2, 1536
T = B * S
NCORES = 8
KD = D // 128           # 6 contraction chunks over D
MF = F // 128           # 12 f-tiles for routed experts
MD = D // 128           # 6 output d-tiles
FS = 384                # shared-expert F-slice per core (4 slices x 2 token halves)
MFS = FS // 128         # 3 f-tiles for shared slice
TH = T // 2             # shared-expert token half
NT = 512                # moving-operand (token) tile

F32 = mybir.dt.float32
COMPUTE_DT = mybir.dt.bfloat16   # matmul operand dtype (bf16: FWL + half DMA)
NP_COMPUTE = np.float32 if COMPUTE_DT == mybir.dt.float32r else ml_dtypes.bfloat16

_cache: dict = {}


def _chunks(total, step=NT):
    out, o = [], 0
    while o < total:
        n = min(step, total - o)
        out.append((o, n))
        o += n
    return out


def _build(C):
    """One SPMD program: routed expert over C tokens + shared slice over TH."""
    nc = bacc.Bacc("TRN2", debug=False)
    xeT = nc.dram_tensor("xeT", [D, C], COMPUTE_DT, kind="ExternalInput")
    wgT = nc.dram_tensor("wgT", [D, F], COMPUTE_DT, kind="ExternalInput")
    wuT = nc.dram_tensor("wuT", [D, F], COMPUTE_DT, kind="ExternalInput")
    wdT = nc.dram_tensor("wdT", [F, D], COMPUTE_DT, kind="ExternalInput")
    xsT = nc.dram_tensor("xsT", [D, TH], COMPUTE_DT, kind="ExternalInput")
    sgT = nc.dram_tensor("sgT", [D, FS], COMPUTE_DT, kind="ExternalInput")
    suT = nc.dram_tensor("suT", [D, FS], COMPUTE_DT, kind="ExternalInput")
    sdT = nc.dram_tensor("sdT", [FS, D], COMPUTE_DT, kind="ExternalInput")
    yeT = nc.dram_tensor("yeT", [D, C], F32, kind="ExternalOutput")
    zT = nc.dram_tensor("zT", [D, TH], F32, kind="ExternalOutput")

    with tile.TileContext(nc) as tc, ExitStack() as ctx:
        wpool = ctx.enter_context(tc.tile_pool(name="w", bufs=1))
        xpool = ctx.enter_context(tc.tile_pool(name="x", bufs=2))
        hpool = ctx.enter_context(tc.tile_pool(name="h", bufs=1))
        spool = ctx.enter_context(tc.tile_pool(name="s", bufs=2))
        opool = ctx.enter_context(tc.tile_pool(name="o", bufs=3))
        pgp = ctx.enter_context(tc.tile_pool(name="pg", bufs=2, space="PSUM"))
        pup = ctx.enter_context(tc.tile_pool(name="pu", bufs=2, space="PSUM"))
        pyp = ctx.enter_context(tc.tile_pool(name="py", bufs=2, space="PSUM"))

        def load_rows(src, width, n, tag):
            tiles = []
            for k in range(n):
                t = wpool.tile([128, width], COMPUTE_DT, tag=f"{tag}{k}")
                nc.sync.dma_start(t[:], src[ts(k, 128), :])
                tiles.append(t)
            return tiles

        wg_sb = load_rows(wgT, F, KD, "wg")
        wu_sb = load_rows(wuT, F, KD, "wu")
        wd_sb = load_rows(wdT, D, MF, "wd")
        sg_sb = load_rows(sgT, FS, KD, "sg")
        su_sb = load_rows(suT, FS, KD, "su")
        sd_sb = load_rows(sdT, D, MFS, "sd")

        # iteration list: routed tiles then shared tiles
        iters = [("R", o, n) for o, n in _chunks(C)] + \
                [("S", o, n) for o, n in _chunks(TH)]

        for ph, o, n in iters:
            x_src = xeT if ph == "R" else xsT
            g_w, u_w = (wg_sb, wu_sb) if ph == "R" else (sg_sb, su_sb)
            mf = MF if ph == "R" else MFS
            out_dst = yeT if ph == "R" else zT

            xt = xpool.tile([128, KD * NT], COMPUTE_DT, tag="xt")
            for k in range(KD):
                nc.sync.dma_start(xt[:, k * NT:k * NT + n],
                                  x_src[ts(k, 128), o:o + n])

            hT = []
            for m in range(mf):
                g = pgp.tile([128, NT], F32, tag="pg")
                u = pup.tile([128, NT], F32, tag="pu")
                for k in range(KD):
                    nc.tensor.matmul(g[:, :n], g_w[k][:, ts(m, 128)],
                                     xt[:, k * NT:k * NT + n],
                                     start=(k == 0), stop=(k == KD - 1))
                for k in range(KD):
                    nc.tensor.matmul(u[:, :n], u_w[k][:, ts(m, 128)],
                                     xt[:, k * NT:k * NT + n],
                                     start=(k == 0), stop=(k == KD - 1))
                sil = spool.tile([128, NT], F32, tag="sil")
                nc.scalar.activation(sil[:, :n], g[:, :n],
                                     mybir.ActivationFunctionType.Sigmoid)
                gs = spool.tile([128, NT], F32, tag="gs")
                nc.vector.tensor_mul(gs[:, :n], sil[:, :n], g[:, :n])
                h = hpool.tile([128, NT], COMPUTE_DT, tag=f"h{m}")
                nc.vector.tensor_mul(h[:, :n], gs[:, :n], u[:, :n])
                hT.append(h)

            for m2 in range(MD):
                y = pyp.tile([128, NT], F32, tag="py")
                d_w = wd_sb if ph == "R" else sd_sb
                for k2 in range(mf):
                    nc.tensor.matmul(y[:, :n], d_w[k2][:, ts(m2, 128)], hT[k2][:, :n],
                                     start=(k2 == 0), stop=(k2 == mf - 1))
                yo = opool.tile([128, NT], F32, tag="yo")
                nc.vector.tensor_copy(yo[:, :n], y[:, :n])
                nc.sync.dma_start(out_dst[ts(m2, 128), o:o + n], yo[:, :n])
    nc.compile()
    return nc


def _router(xf, w_router, expert_bias):
    """Replicates the reference router. f64 for stable top-k ordering,
    f32 softmax (same formula as jax.nn.softmax) for the weights."""
    logits = xf.astype(np.float64) @ w_router.T.astype(np.float64)
    l32 = (xf @ w_router.T).astype(np.float32)
    m = l32.max(-1, keepdims=True)
    e32 = np.exp(l32 - m)
    scores = e32 / e32.sum(-1, keepdims=True)
    e64 = np.exp(logits - logits.max(-1, keepdims=True))
    sel = e64 / e64.sum(-1, keepdims=True) + expert_bias.astype(np.float64)[None, :]
    top_idx = np.argsort(-sel, axis=-1, kind="stable")[:, :TOPK]
    top_s = np.take_along_axis(scores, top_idx, axis=-1)
    top_s = top_s / (top_s.sum(-1, keepdims=True) + 1e-9)
    return top_idx, top_s


def kernel(x, w_router, expert_bias, Wg, Wu, Wd, sg, su, sd):
    x = np.asarray(x); w_router = np.asarray(w_router)
    expert_bias = np.asarray(expert_bias)
    Wg = np.asarray(Wg); Wu = np.asarray(Wu); Wd = np.asarray(Wd)
    sg = np.asarray(sg); su = np.asarray(su); sd = np.asarray(sd)
    xf = x.reshape(-1, D).astype(np.float32)

    top_idx, top_s = _router(xf, w_router, expert_bias)

    idxs, ws = [], []
    for e in range(E):
        hit = (top_idx == e)
        tok = np.nonzero(hit.any(-1))[0]
        idxs.append(tok)
        ws.append(top_s[tok][hit[tok]])
    cmax = max(len(i) for i in idxs)
    C = max(512, -(-cmax // 128) * 128)

    key = C
    if key not in _cache:
        _cache[key] = _build(C)
    nc = _cache[key]

    cast = lambda a: np.ascontiguousarray(a, dtype=np.float32).astype(NP_COMPUTE)
    in_maps = []
    for e in range(E):
        xeT = np.zeros((D, C), np.float32)
        xeT[:, :len(idxs[e])] = xf[idxs[e]].T
        th, fq = e // 4, e % 4
        in_maps.append({
            "xeT": cast(xeT),
            "wgT": cast(Wg[e].T), "wuT": cast(Wu[e].T), "wdT": cast(Wd[e].T),
            "xsT": cast(xf[th * TH:(th + 1) * TH].T),
            "sgT": cast(sg[fq * FS:(fq + 1) * FS].T),
            "suT": cast(su[fq * FS:(fq + 1) * FS].T),
            "sdT": cast(sd[:, fq * FS:(fq + 1) * FS].T),
        })

    res = run_bass_kernel_spmd(nc, in_maps, core_ids=list(range(NCORES)))

    out = np.zeros((T, D), np.float32)
    for e in range(E):
        ye = res.results[e]["yeT"].T[:len(idxs[e])]
        out[idxs[e]] += ws[e][:, None] * ye
        th = e // 4
        out[th * TH:(th + 1) * TH] += res.results[e]["zT"].T
    return out.reshape(B, S, D).astype(x.dtype)
